# revision 23
# baseline (speedup 1.0000x reference)
"""Multi-head attention (B=2, S=2048, D=1024, H=16) on 8 TRN2 NeuronCores.

Sharding: core = (batch b, head-group g): 2 batches x 4 groups of 4 heads.
Each core computes its group's QKV projections, attention, and a partial
output projection; the host sums the 4 partials per batch and adds the
exact bias constant (bv @ Wo.T + bo). bq/bk are applied on device.

Matmul dtype is configurable (default fp16):
  fp16: operands stored/shipped as float16 (10-bit mantissa), 1 PE
        cycle/row + fast weight load; fp32 PSUM accumulation.
        ~1.3e-3 max rel error at the same speed as bf16.
  bf16: same rate, ~8e-3 max rel error.
  f32r: fp32 data rounded to the PE's TF32-like fast format, 2 cycles/row.
        ~5.5e-4 max rel error, ~1.5x slower.
The softmax normalization chain stays in f32/f32r in every mode so the
denominator carries no 16-bit error.

Per-core layout:
  xT [D, S] host-transposed inputs; QT/KT [JJ, S] head-dim-major so scores
  come out keys-on-partitions (S.T tiles) and the key-axis softmax reduction
  happens inside the P.T @ V' matmul via a ones-column appended to V'
  (PSUM row 64 of the PV output accumulates the softmax denominator).
  V' stationaries are padded to 128 columns to keep fast weight loads.
  OT [JJ, S] normalized attention output feeds the output projection as
  lhsT, giving the partial output in natural [S, D] layout.
"""
from contextlib import ExitStack

import numpy as np

# Problem constants (hardcoded per harness contract).
B, S, D, H = 2, 2048, 1024, 16
HD = D // H          # 64
N_CORES = 8
GROUPS = N_CORES // B    # 4
H_LOC = H // GROUPS      # 4 heads per core
JJ = H_LOC * HD          # 256
P = 128

MM_DT = "fp16"  # "fp16" | "bf16" | "f32r"


def build_mha(s=S, d=D, h_loc=H_LOC, hd=HD, chunk=1024, nf=512, mm_dt=MM_DT):
    """Build + compile the per-core Bass program."""
    import concourse.bacc as bacc
    import concourse.tile as tile
    from concourse import mybir

    f32 = mybir.dt.float32
    f32r = mybir.dt.float32r
    _two_byte = {"bf16": mybir.dt.bfloat16, "fp16": mybir.dt.float16}
    mdt = _two_byte.get(mm_dt, f32r)
    in_dt = _two_byte.get(mm_dt, f32)  # DRAM dtype of x / weights
    Exp = mybir.ActivationFunctionType.Exp
    Ident = mybir.ActivationFunctionType.Identity

    jj = h_loc * hd
    hd1 = hd + 1
    ktd = d // P
    njt = (jj + P - 1) // P
    st_n = s // P
    chunk = min(chunk, s)
    nf = min(nf, chunk)
    n_ch = s // chunk
    nfc = chunk // nf
    ndo = (d + nf - 1) // nf
    pc = min(512, s)

    nc = bacc.Bacc("TRN2", target_bir_lowering=False, debug=False)

    xq = nc.dram_tensor("xq", [d, s], in_dt, kind="ExternalInput").ap()
    xk = nc.dram_tensor("xk", [d, s], in_dt, kind="ExternalInput").ap()
    xv = nc.dram_tensor("xv", [d, s], in_dt, kind="ExternalInput").ap()
    wq = nc.dram_tensor("wq", [d, jj], in_dt, kind="ExternalInput").ap()
    wk = nc.dram_tensor("wk", [d, jj], in_dt, kind="ExternalInput").ap()
    wv = nc.dram_tensor("wv", [d, jj], in_dt, kind="ExternalInput").ap()
    wo = nc.dram_tensor("wo", [jj, d], in_dt, kind="ExternalInput").ap()
    bqp = nc.dram_tensor("bqp", [jj, 1], f32, kind="ExternalInput").ap()
    bkp = nc.dram_tensor("bkp", [jj, 1], f32, kind="ExternalInput").ap()
    out = nc.dram_tensor("out", [s, d], f32, kind="ExternalOutput").ap()

    with tile.TileContext(nc) as tc, ExitStack() as ctx:
        persist = ctx.enter_context(tc.tile_pool(name="persist", bufs=1))

        qt_sb = [persist.tile([P, s], mdt, name=f"qt{j}", tag=f"qt{j}") for j in range(njt)]
        kt_sb = [persist.tile([P, s], mdt, name=f"kt{j}", tag=f"kt{j}") for j in range(njt)]
        ot_sb = [persist.tile([P, s], mdt, name=f"ot{j}", tag=f"ot{j}") for j in range(njt)]
        # padded per-(seq-tile, head) PV stationaries: [V_h | ones | zeros]
        v_sb = [[persist.tile([P, P], mdt, name=f"v{t}_{h}", tag=f"v{t}_{h}")
                 for h in range(h_loc)] for t in range(st_n)]
        wq_r = [persist.tile([P, jj], mdt, name=f"wqr{k}", tag=f"wqr{k}") for k in range(ktd)]
        wk_r = [persist.tile([P, jj], mdt, name=f"wkr{k}", tag=f"wkr{k}") for k in range(ktd)]
        wv_r = [persist.tile([P, jj], mdt, name=f"wvr{k}", tag=f"wvr{k}") for k in range(ktd)]
        wo_r = [persist.tile([P, d], mdt, name=f"wor{j}", tag=f"wor{j}") for j in range(njt)]
        bq_sb = persist.tile([P, njt], f32, name="bq_sb", tag="bq_sb")
        bk_sb = persist.tile([P, njt], f32, name="bk_sb", tag="bk_sb")
        ones_v = persist.tile([P, 1], f32, name="ones_v", tag="ones_v")
        ones_h = persist.tile([1, hd], f32, name="ones_h", tag="ones_h")
        ones_hr = persist.tile([1, hd], f32r, name="ones_hr", tag="ones_hr")

        nc.vector.memset(ones_v[:], 1.0)
        nc.vector.memset(ones_h[:], 1.0)
        nc.vector.tensor_copy(ones_hr[:], ones_h[:])
        for j in range(njt):
            nc.scalar.dma_start(bq_sb[:, j:j + 1], bqp[j * P:(j + 1) * P, :])
            nc.scalar.dma_start(bk_sb[:, j:j + 1], bkp[j * P:(j + 1) * P, :])

        # ---- weights ----
        # wk first: the K-projection is the first consumer, so its weight
        # tiles and xk tiles (loaded right after, in load_xr) lead the DMA
        # queue; wq/wv/wo follow on the scalar-engine HWDGE queue so they
        # don't delay the critical path.
        if mm_dt in _two_byte:
            for k in range(ktd):
                nc.sync.dma_start(wk_r[k][:], wk[k * P:(k + 1) * P, :])
            for k in range(ktd):
                nc.scalar.dma_start(wq_r[k][:], wq[k * P:(k + 1) * P, :])
                nc.scalar.dma_start(wv_r[k][:], wv[k * P:(k + 1) * P, :])
            for j in range(njt):
                nc.scalar.dma_start(wo_r[j][:], wo[j * P:(j + 1) * P, :])
        else:
            with tc.tile_pool(name="wstage", bufs=3) as wstage:
                for k in range(ktd):
                    for nm, dr, dst in (("q", wq, wq_r), ("k", wk, wk_r), ("v", wv, wv_r)):
                        wtmp = wstage.tile([P, jj], f32, name=f"w{nm}s{k}", tag="wst")
                        nc.sync.dma_start(wtmp[:], dr[k * P:(k + 1) * P, :])
                        nc.vector.tensor_copy(dst[k][:], wtmp[:])
                for j in range(njt):
                    wtmp = wstage.tile([P, d], f32, name=f"wos{j}", tag="wost")
                    nc.sync.dma_start(wtmp[:], wo[j * P:(j + 1) * P, :])
                    nc.vector.tensor_copy(wo_r[j][:], wtmp[:])

        # ---- projections ----
        with tc.tile_pool(name="xpool", bufs=3) as xpool, \
             tc.tile_pool(name="xrpool", bufs=ktd) as xrpool, \
             tc.tile_pool(name="ppsum", bufs=3, space="PSUM") as ppsum:

            def load_xr(xdr):
                tiles = []
                for k in range(ktd):
                    if mm_dt in ("bf16", "fp16"):
                        xr = xrpool.tile([P, s], mdt, name=f"xr{k}", tag="xr")
                        nc.sync.dma_start(xr[:], xdr[k * P:(k + 1) * P, :])
                    else:
                        xs = xpool.tile([P, s], f32, name=f"xs{k}", tag="xs")
                        nc.sync.dma_start(xs[:], xdr[k * P:(k + 1) * P, :])
                        xr = xrpool.tile([P, s], mdt, name=f"xr{k}", tag="xr")
                        nc.vector.tensor_copy(xr[:], xs[:])
                    tiles.append(xr)
                return tiles

            for nm, xdr, w_r, dst, bias_sb, scale in (
                ("k", xk, wk_r, kt_sb, bk_sb, 1.0),
                ("q", xq, wq_r, qt_sb, bq_sb, float(1.0 / np.sqrt(hd))),
            ):
                xr_t = load_xr(xdr)
                ncp = s // pc
                for j in range(njt):
                    # k-outer / c-inner so each weight stationary load serves
                    # ncp moving streams
                    pps = [ppsum.tile([P, pc], f32, name=f"pp{nm}{j}_{c}", tag="pp",
                                      bufs=ncp + 1)
                           for c in range(ncp)]
                    for k in range(ktd):
                        for c in range(ncp):
                            nc.tensor.matmul(
                                pps[c][:], w_r[k][:, j * P:(j + 1) * P],
                                xr_t[k][:, c * pc:(c + 1) * pc],
                                start=(k == 0), stop=(k == ktd - 1))
                    for c in range(ncp):
                        nc.scalar.activation(
                            dst[j][:, c * pc:(c + 1) * pc], pps[c][:], Ident,
                            bias=bias_sb[:, j:j + 1], scale=scale)

            # V' padded stationaries
            xr_t = load_xr(xv)
            for t in range(st_n):
                pv = ppsum.tile([P, jj], f32, name=f"pv{t}", tag="pv", bufs=3)
                for k in range(ktd):
                    nc.tensor.matmul(pv[:], xr_t[k][:, t * P:(t + 1) * P],
                                     wv_r[k][:], start=(k == 0), stop=(k == ktd - 1))
                for h in range(h_loc):
                    vt = v_sb[t][h]
                    nc.vector.tensor_copy(vt[:, 0:hd], pv[:, h * hd:(h + 1) * hd])
                    nc.vector.tensor_copy(vt[:, hd:hd1], ones_v[:])
                    if hd1 < P:
                        nc.gpsimd.memset(vt[:, hd1:P], 0.0)

        # ---- attention ----
        # Per head, two passes over the full sequence:
        #   pass 1: scores.T tiles (one KT stationary load per seq-tile, s/nf
        #           moving streams) -> exp over [128, s] -> PT tiles
        #   pass 2: PV accumulation (one V' stationary load per seq-tile,
        #           s/nf moving streams) -> [128, s] psum, row hd = denominators
        # PSUM: sp [128,s] (s/512 banks) + otp [128,s] -> 8 banks total.
        ec = min(1024, s)          # exp / score-psum chunk of the q axis
        nec = s // ec
        efc = ec // nf
        with tc.tile_pool(name="spsum", bufs=2, space="PSUM") as spsum, \
             tc.tile_pool(name="opsum", bufs=1, space="PSUM") as opsum, \
             tc.tile_pool(name="ptpool", bufs=3 * nec + 2) as ptpool, \
             tc.tile_pool(name="npool", bufs=2) as npool:
            pending_norm = None
            for h in range(h_loc):
                jt = (h * hd) // P
                off = (h * hd) % P
                otp = opsum.tile([P, s], f32, name=f"otp{h}", tag="otp")
                pts = {}

                def scores(t):
                    for e in range(nec):
                        sp = spsum.tile([P, ec], f32, name=f"sp{h}_{t}_{e}", tag="sp")
                        for f in range(efc):
                            q0 = e * ec + f * nf
                            nc.tensor.matmul(
                                sp[:, f * nf:(f + 1) * nf],
                                kt_sb[jt][off:off + hd, t * P:(t + 1) * P],
                                qt_sb[jt][off:off + hd, q0:q0 + nf],
                                start=True, stop=True)
                        pt = ptpool.tile([P, ec], mdt, name=f"pt{h}_{t}_{e}", tag="pt")
                        nc.scalar.activation(pt[:], sp[:], Exp)
                        pts[t, e] = pt

                def pv(t):
                    for e in range(nec):
                        for f in range(efc):
                            q0 = e * ec + f * nf
                            nc.tensor.matmul(
                                otp[:, q0:q0 + nf],
                                v_sb[t][h][:],
                                pts[t, e][:, f * nf:(f + 1) * nf],
                                start=(t == 0), stop=(t == st_n - 1))
                        del pts[t, e]

                # software-pipeline: scores(t+1) emitted before pv(t); the
                # previous head's normalize is emitted into this head's
                # scores stream so its broadcast matmuls don't stall the PE.
                scores(0)
                for t in range(1, st_n):
                    scores(t)
                    if t == 2 and pending_norm is not None:
                        pending_norm()
                        pending_norm = None
                    pv(t - 1)
                pv(st_n - 1)
                # Evict the PV accumulator to SBUF with one copy so the PSUM
                # frees for the next head immediately; the normalize chain
                # (rowsum broadcast, reciprocal, scale) runs off the critical
                # path, chunked so the output projection can start early.
                rs_r = npool.tile([1, s], f32r, name=f"rs{h}", tag="rs")
                nc.scalar.activation(rs_r[:], otp[hd:hd1, :],
                                     mybir.ActivationFunctionType.Copy)
                ob = npool.tile([hd, s], f32, name=f"obuf{h}", tag="obuf")
                nc.vector.tensor_copy(ob[:], otp[0:hd, :])

                # the last head's normalize gates the output projection, so
                # chunk it finer there to release early columns sooner
                cw = nf if h == h_loc - 1 else ec
                cfc = cw // nf

                def norm(ob=ob, rs_r=rs_r, jt=jt, off=off, h=h, cw=cw, cfc=cfc):
                    for e in range(s // cw):
                        bp = spsum.tile([hd, cw], f32, name=f"bp{h}_{e}", tag="sp")
                        for f in range(cfc):
                            q0 = e * cw + f * nf
                            nc.tensor.matmul(bp[:, f * nf:(f + 1) * nf],
                                             ones_hr[:], rs_r[:, q0:q0 + nf],
                                             start=True, stop=True)
                        binv = npool.tile([hd, cw], f32,
                                          name=f"binv{h}_{e}", tag="binv")
                        nc.vector.reciprocal(binv[:], bp[:])
                        nc.vector.tensor_mul(
                            ot_sb[jt][off:off + hd, e * cw:(e + 1) * cw],
                            ob[0:hd, e * cw:(e + 1) * cw], binv[:])

                if pending_norm is not None:  # small-config fallback
                    pending_norm()
                pending_norm = norm
            pending_norm()

        # ---- output projection (natural layout) ----
        with tc.tile_pool(name="fpsum", bufs=2, space="PSUM") as fpsum, \
             tc.tile_pool(name="fout", bufs=2) as fout:
            for t in range(st_n):
                po = fpsum.tile([P, d], f32, name=f"po{t}", tag="po")
                for njx in range(ndo):
                    for j in range(njt):
                        nc.tensor.matmul(
                            po[:, njx * nf:(njx + 1) * nf],
                            ot_sb[j][:, t * P:(t + 1) * P],
                            wo_r[j][:, njx * nf:(njx + 1) * nf],
                            start=(j == 0), stop=(j == njt - 1))
                ob = fout.tile([P, d], f32, name=f"ob{t}", tag="ob")
                nc.scalar.copy(ob[:], po[:])
                nc.sync.dma_start(out[t * P:(t + 1) * P, :], ob[:])

    nc.compile()
    return nc


_NC_CACHE = {}


def _get_nc():
    key = MM_DT
    if key not in _NC_CACHE:
        _NC_CACHE[key] = build_mha(mm_dt=key)
    return _NC_CACHE[key]


def build_in_maps(inputs, mm_dt=MM_DT):
    if mm_dt == "bf16":
        import ml_dtypes
        xdt = ml_dtypes.bfloat16
    elif mm_dt == "fp16":
        xdt = np.float16
    else:
        xdt = np.float32

    q = np.asarray(inputs["query"], np.float32)
    k = np.asarray(inputs.get("key_", inputs.get("key")), np.float32)
    v = np.asarray(inputs["value"], np.float32)
    Wq = np.asarray(inputs["Wq"], np.float32)
    Wk = np.asarray(inputs["Wk"], np.float32)
    Wv = np.asarray(inputs["Wv"], np.float32)
    Wo = np.asarray(inputs["Wo"], np.float32)
    bq = np.asarray(inputs["bq"], np.float32)
    bk = np.asarray(inputs["bk"], np.float32)

    sc = np.float32(1.0 / np.sqrt(HD))
    qT = [np.ascontiguousarray(q[b].T).astype(xdt) for b in range(B)]
    kT = [np.ascontiguousarray(k[b].T).astype(xdt) for b in range(B)]
    vT = [np.ascontiguousarray(v[b].T).astype(xdt) for b in range(B)]
    WqT = np.ascontiguousarray(Wq.T)
    WkT = np.ascontiguousarray(Wk.T)
    WvT = np.ascontiguousarray(Wv.T)

    in_maps = []
    for core in range(N_CORES):
        b, g = divmod(core, GROUPS)
        sl = slice(g * JJ, (g + 1) * JJ)
        in_maps.append({
            "xq": qT[b],
            "xk": kT[b],
            "xv": vT[b],
            "wq": np.ascontiguousarray(WqT[:, sl]).astype(xdt),
            "wk": np.ascontiguousarray(WkT[:, sl]).astype(xdt),
            "wv": np.ascontiguousarray(WvT[:, sl]).astype(xdt),
            "wo": np.ascontiguousarray(Wo[:, sl].T).astype(xdt),
            "bqp": np.ascontiguousarray((bq[sl] * sc)[:, None]),
            "bkp": np.ascontiguousarray(bk[sl][:, None]),
        })
    return in_maps


def combine_outputs(results, inputs):
    Wo = np.asarray(inputs["Wo"], np.float32)
    bv = np.asarray(inputs["bv"], np.float32)
    bo = np.asarray(inputs["bo"], np.float32)
    const = bv @ Wo.T + bo  # exact host-side bias correction
    outp = np.empty((B, S, D), np.float32)
    for b in range(B):
        acc = results[b * GROUPS]["out"].astype(np.float32).copy()
        for g in range(1, GROUPS):
            acc += results[b * GROUPS + g]["out"]
        outp[b] = acc + const[None, :]
    return outp


def kernel(**inputs):
    import time
    from concourse.bass_utils import run_bass_kernel_spmd

    nc = _get_nc()
    in_maps = build_in_maps(inputs)
    last_err = None
    for attempt in range(3):
        try:
            res = run_bass_kernel_spmd(nc, in_maps, list(range(N_CORES)))
            return combine_outputs(res.results, inputs)
        except Exception as e:  # transient device wedge: retry
            last_err = e
            try:
                # poke each core with a trivial op to clear transient
                # exec-unit state before retrying
                import jax
                import jax.numpy as jnp
                for dvc in jax.devices()[:N_CORES]:
                    jax.device_put(jnp.zeros((8, 8)), dvc).block_until_ready()
            except Exception:
                pass
            time.sleep(5.0 * (attempt + 1))
    raise last_err
